# revision 20
# baseline (speedup 1.0000x reference)
"""Multi-head attention with KV-cache append, sharded over 8 trn2 NeuronCores.

Reference computation (fp32):
    qkv = x @ w_qkv + b_qkv                  # x [2,256,1024]
    q,k_new,v_new -> [B,H,N,D] (H=16, D=64)
    k_all = cat(past_k, k_new); v_all = cat(past_v, v_new)   # [B,H,8448,64]
    out = softmax(q k^T / sqrt(D)) @ v_all   # non-causal
    return out.merge_heads @ w_proj + b_proj

Sharding: tensor-parallel over heads. Core c owns heads [2c, 2c+1]:
  - past_k/past_v sharded by head and converted to bf16 on the host;
    past_k pre-transposed to [B,2,D,L] and past_v re-blocked to
    [B,2,128,L/128,D] so every DMA reads fully contiguous per-partition
    runs. On-device, v tiles get a ones column appended, so the attn@v
    matmul also produces the softmax denominator row for free.
  - w_qkv column-split / w_proj row-split per head (bf16); each core
    computes a full-shape bf16 partial of the output projection; host sums
    partials in f64 + b_proj.

Device kernel per core (all engines via the Tile framework):
  q^T/k_new^T/v_new projections on PE, flash-style streaming over the KV
  cache: scores^T block [128l, 256n] on PE (two heads row-group concurrent)
  -> exp split between ACT (table exp, scale folded) and DVE (Schraudolph
  bf16 bit-trick: one tensor_scalar) -> attn@v accumulation in PSUM with
  the ones-column producing the softmax denominator row. Epilogue: per-head
  output projection, per-token denominator scaling (softmax normalization
  commutes with the projection within a head) on DVE, cross-head sum on
  gpsimd, bf16 output partials.
"""

import sys
for _p in ("/opt/trn_rl_repo", "/root/.axon_site/_ro/trn_rl_repo"):
    if _p not in sys.path:
        sys.path.append(_p)

from contextlib import ExitStack

import numpy as np

import concourse.bass as bass
import concourse.tile as tile
from concourse import bacc, mybir
from concourse.bass_utils import run_bass_kernel_spmd
from concourse.masks import make_identity

N_CORES = 8
B, N, DIM = 2, 256, 1024
H, D = 16, 64
L = 8192
HL = H // N_CORES          # 2 heads per core
DL = HL * D                # 128 local head dims
SCALE = D ** -0.5
T = B * N                  # 512 tokens
CC = DIM // 128            # 8 contraction chunks of the model dim
F32 = mybir.dt.float32
VP = D + 1                 # v padded with ones column
NBLK = 128                 # kv-cache block (l) per scores matmul
PSB = 4                    # l-blocks per PSUM superblock -> [128, PSB*N]
DSB = 4096                 # l per DMA superblock

# bf16 data path: halves HBM traffic vs f32 and streams the PE at 1
# cycle/row at any moving size (fp32r pays 4x below 256). The ~0.4% bf16
# rounding noise is far inside the 2e-2 harness tolerance.
MMD = mybir.dt.bfloat16
U16 = mybir.dt.uint16

# Softmax exp is ~56us of ACT work if done entirely on the scalar engine --
# the single largest serial engine cost. Offload a fraction of the exp tiles
# to the DVE using the Schraudolph bit trick in bf16:
#   bf16_bits(exp(s*SCALE)) ~= round(s * EXP_A + EXP_B)
# (one tensor_scalar mult+add, f32 PSUM in -> uint16 out, bitcast to bf16).
# Max rel err ~3.3%, rms ~1.8%, pseudo-random across keys -> averages out in
# the softmax weighted mean (common-mode cancels in the ratio entirely).
EXP_A = float(SCALE * np.log2(np.e) * 128.0)
EXP_B = 16250.5
# of every 16 exp tiles, these slot indices run on DVE (rest on ACT)
DVE_SLOTS = frozenset((1, 3, 5, 7, 9, 11, 13))


def _emit(ctx: ExitStack, tc: tile.TileContext, aps: dict):
    nc = tc.nc
    t_x, t_k, t_v = aps["x_t"], aps["k_t"], aps["v_pad"]
    t_w, t_bq, t_bv = aps["w_loc"], aps["b_q"], aps["b_v"]
    t_wp, t_out = aps["wp_loc"], aps["out"]
    ablate = aps.get("ablate", "")

    singles = ctx.enter_context(tc.tile_pool(name="singles", bufs=1))
    kpool = ctx.enter_context(tc.tile_pool(name="kpool", bufs=2))
    vpool = ctx.enter_context(tc.tile_pool(name="vpool", bufs=4))
    apool = ctx.enter_context(tc.tile_pool(name="apool", bufs=4))
    epool = ctx.enter_context(tc.tile_pool(name="epool", bufs=4))
    opool = ctx.enter_context(tc.tile_pool(name="opool", bufs=6))
    rpool = ctx.enter_context(tc.tile_pool(name="rpool", bufs=8))
    scp = ctx.enter_context(tc.tile_pool(name="scp", bufs=3, space="PSUM"))
    acp = ctx.enter_context(tc.tile_pool(name="acp", bufs=2, space="PSUM"))

    Exp = mybir.ActivationFunctionType.Exp
    Ident = mybir.ActivationFunctionType.Identity

    def body():
        # ---- load x^T, weights, biases ----
        # x/w split into halves so the qkv matmuls (which consume one
        # cc-chunk at a time) start as soon as the first half lands
        x_sb = singles.tile([128, CC, T], MMD, tag="x")
        xr = t_x.rearrange("(cc p) t -> p cc t", p=128)
        nc.scalar.dma_start(out=x_sb[:, 0:CC // 2], in_=xr[:, 0:CC // 2])
        nc.scalar.dma_start(out=x_sb[:, CC // 2:CC], in_=xr[:, CC // 2:CC])
        w_sb = singles.tile([128, CC, 3 * DL], MMD, tag="w")
        wr = t_w.rearrange("(cc p) d -> p cc d", p=128)
        nc.scalar.dma_start(out=w_sb[:, 0:CC // 2], in_=wr[:, 0:CC // 2])
        nc.scalar.dma_start(out=w_sb[:, CC // 2:CC], in_=wr[:, CC // 2:CC])
        wp_sb = singles.tile([D, HL, DIM], MMD, tag="wp")
        nc.scalar.dma_start(out=wp_sb, in_=t_wp)
        bqk_sb = singles.tile([DL, 2], F32, tag="bqk")
        nc.scalar.dma_start(out=bqk_sb, in_=t_bq)
        bq_sb, bk_sb = bqk_sb[:, 0:1], bqk_sb[:, 1:2]
        bv_sb = singles.tile([128, DL], F32, tag="bv")
        nc.gpsimd.dma_start(
            out=bv_sb,
            in_=bass.AP(tensor=t_bv.tensor, offset=0, ap=[[0, 128], [1, DL]]),
        )
        ident = singles.tile([128, 128], F32, tag="ident")
        make_identity(nc, ident)
        ones_sb = singles.tile([128, 1], F32, tag="ones")
        nc.vector.memset(ones_sb, 1.0)
        # touch Exp once while ACT is otherwise idle: pulls the ~2.7us
        # ACT_TABLE_LOAD for the exp table set into the startup window
        # instead of stalling the first real exp of the pipeline
        warm = singles.tile([128, 1], F32, tag="warm")
        nc.scalar.activation(warm, ones_sb, Exp)

        if ablate == "dmaonly":
            for b in range(B):
                for dsb in range(L // DSB):
                    NB = DSB // NBLK
                    kt = kpool.tile([128, DSB], MMD, tag="k")
                    nc.sync.dma_start(
                        out=kt,
                        in_=t_k[b].rearrange("h d l -> (h d) l")[:, dsb * DSB:(dsb + 1) * DSB])
                    for h in range(HL):
                        vt = vpool.tile([128, NB, VP], MMD, tag="v", name=f"vt{h}")
                        veng = nc.scalar if h == 0 else nc.sync
                        veng.dma_start(
                            out=vt,
                            in_=t_v[b, h][:, dsb * NB:(dsb + 1) * NB, :])
            for g in range(4):
                for ech in range(2):
                    so = opool.tile([128, 512], MMD, tag="o")
                    nc.vector.tensor_copy(so, x_sb[:, g, :])
                    nc.sync.dma_start(
                        out=t_out[g * 128:(g + 1) * 128,
                                  ech * 512:(ech + 1) * 512],
                        in_=so)
            return

        at0 = None
        if ablate in ("noact", "nopev"):
            at0 = singles.tile([128, PSB * N], MMD, tag="at0")
            nc.vector.memset(at0, 0.001)
        exp_cnt = [0]

        # ---- qkv projections ----
        # q^T, k_new^T in [d_local, token] layout (d on partitions)
        psq = scp.tile([128, T], F32, tag="sc")
        for cc in range(CC):
            nc.tensor.matmul(psq, w_sb[:, cc, 0:DL], x_sb[:, cc, :],
                             start=(cc == 0), stop=(cc == CC - 1))
        q_sb = singles.tile([DL, T], MMD, tag="q")
        nc.scalar.activation(q_sb, psq, Ident, bias=bq_sb)

        psk = scp.tile([128, T], F32, tag="sc")
        for cc in range(CC):
            nc.tensor.matmul(psk, w_sb[:, cc, DL:2 * DL], x_sb[:, cc, :],
                             start=(cc == 0), stop=(cc == CC - 1))
        k_sb = singles.tile([DL, T], MMD, tag="kn")
        nc.scalar.activation(k_sb, psk, Ident, bias=bk_sb)

        # v_new in [token, d_local] layout, padded with the ones column
        vn_sb = {}
        for nch in range(T // 128):  # 4 chunks of 128 tokens
            psv = acp.tile([128, DL], F32, tag="ac")
            for cc in range(CC):
                nc.tensor.matmul(psv, x_sb[:, cc, nch * 128:(nch + 1) * 128],
                                 w_sb[:, cc, 2 * DL:3 * DL],
                                 start=(cc == 0), stop=(cc == CC - 1))
            for h in range(HL):
                vn = singles.tile([128, VP], MMD, tag=f"vn{nch}_{h}")
                nc.vector.tensor_add(vn[:, 0:D], psv[:, h * D:(h + 1) * D],
                                     bv_sb[:, h * D:(h + 1) * D])
                nc.vector.tensor_copy(vn[:, D:VP], ones_sb)
                vn_sb[(nch, h)] = vn

        # ---- streaming attention over the KV cache ----
        for b in range(B):
            acc = {}
            for h in range(HL):
                acc[(b, h)] = acp.tile([VP, N], F32, tag="ac", name=f"acc{b}{h}")

            # software pipeline: the attn@v of superblock i is emitted after
            # the scores of superblock i+1 so the (strict-FIFO) PE never
            # stalls waiting for the exp of the tile it just produced
            pend = []

            def flush_attnv():
                while pend:
                    p_ats, p_vts, base = pend.pop(0)
                    for j in range(PSB):
                        for h in range(HL):
                            if ablate == "nopev":
                                continue
                            nc.tensor.matmul(
                                acc[(b, h)],
                                p_vts[h][:, base % (DSB // NBLK) + j, :],
                                p_ats[h][:, j * N:(j + 1) * N],
                                start=(base + j == 0), stop=False,
                                skip_group_check=True)
            for dsb in range(L // DSB):  # DMA superblocks
                l0 = dsb * DSB
                NB = DSB // NBLK
                kt = kpool.tile([128, DSB], MMD, tag="k")
                nc.sync.dma_start(
                    out=kt,
                    in_=t_k[b].rearrange("h d l -> (h d) l")[:, l0:l0 + DSB])
                vts = {}
                for h in range(HL):
                    # v is host-laid-out as [128, L//128, D+1] with the ones
                    # column baked in: partition = l within a 128-block ->
                    # one fully contiguous per-partition DMA run (>=512B, no
                    # SDMA read-modify-write), and no device-side ones copy.
                    # The two heads ride different DMA queues.
                    vt = vpool.tile([128, NB, VP], MMD, tag="v", name=f"vt{h}")
                    veng = nc.scalar if h == 0 else nc.sync
                    veng.dma_start(
                        out=vt,
                        in_=t_v[b, h][:, dsb * NB:(dsb + 1) * NB, :])
                    vts[h] = vt
                for psb in range(DSB // (PSB * NBLK)):
                    # the two heads' scores matmuls sit on disjoint PE
                    # row-groups (partitions 0-63 / 64-127): interleaving
                    # them lets the array run both concurrently
                    pss, ats = {}, {}
                    for h in range(HL):
                        pss[h] = scp.tile([128, PSB * N], F32, tag="sc",
                                          name=f"ps{h}")
                    for j in range(PSB):
                        lo = psb * PSB * NBLK + j * NBLK
                        for h in range(HL):
                            nc.tensor.matmul(
                                pss[h][:, j * N:(j + 1) * N],
                                kt[h * D:(h + 1) * D, lo:lo + NBLK],
                                q_sb[h * D:(h + 1) * D, b * N:(b + 1) * N],
                                start=True, stop=True)
                    for h in range(HL):
                        if ablate == "noact":
                            ats[h] = at0
                            continue
                        slot = exp_cnt[0] % 16
                        exp_cnt[0] += 1
                        if slot in DVE_SLOTS:
                            atu = apool.tile([128, PSB * N], U16, tag="a",
                                             name=f"at{h}")
                            nc.vector.tensor_scalar(
                                atu, pss[h], EXP_A, EXP_B,
                                mybir.AluOpType.mult, mybir.AluOpType.add)
                            ats[h] = atu.bitcast(MMD)
                        else:
                            at = apool.tile([128, PSB * N], MMD, tag="a",
                                            name=f"at{h}")
                            nc.scalar.activation(at, pss[h], Exp, scale=SCALE)
                            ats[h] = at
                    prev = pend
                    pend = [(ats, vts, dsb * (DSB // NBLK) + psb * PSB)]
                    while prev:
                        p_ats, p_vts, base = prev.pop(0)
                        for j in range(PSB):
                            for h in range(HL):
                                if ablate == "nopev":
                                    continue
                                nc.tensor.matmul(
                                    acc[(b, h)],
                                    p_vts[h][:, base % (DSB // NBLK) + j, :],
                                    p_ats[h][:, j * N:(j + 1) * N],
                                    start=(base + j == 0), stop=False,
                                    skip_group_check=True)
            flush_attnv()
            # new tokens (the appended k_new/v_new of this batch)
            for h in range(HL):
                ps = scp.tile([128, 2 * N], F32, tag="sc")
                for j in range(2):
                    nc.tensor.matmul(
                        ps[:, j * N:(j + 1) * N],
                        k_sb[h * D:(h + 1) * D,
                                b * N + j * NBLK:b * N + (j + 1) * NBLK],
                        q_sb[h * D:(h + 1) * D, b * N:(b + 1) * N],
                        start=True, stop=True)
                at = apool.tile([128, 2 * N], MMD, tag="a")
                nc.scalar.activation(at, ps, Exp, scale=SCALE)
                for j in range(2):
                    nc.tensor.matmul(
                        acc[(b, h)], vn_sb[(b * 2 + j, h)],
                        at[:, j * N:(j + 1) * N],
                        start=(ablate == "nopev" and j == 0),
                        stop=(j == 1), skip_group_check=True)

            # ---- per-batch epilogue: denominators, per-head projection,
            # normalize+sum (softmax normalization commutes with the
            # projection within a head) ----
            uns, recips = {}, {}
            for h in range(HL):
                # attn-out (unnormalized) to SBUF bf16 for the oproj lhsT
                un = epool.tile([D, N], MMD, tag="un")
                nc.vector.tensor_copy(un, acc[(b, h)][0:D, :])
                uns[h] = un
                # denominator row stays f32: copy out (partition-aligned at
                # row 64), transpose each 128-token chunk to a per-partition
                # column, reciprocal
                den = epool.tile([VP, N], F32, tag="den")
                nc.vector.tensor_copy(den[D:VP, :], acc[(b, h)][D:VP, :])
                for tch in range(N // 128):
                    tp = scp.tile([128, 1], F32, tag="sc")
                    nc.tensor.transpose(
                        tp, den[D:VP, tch * 128:(tch + 1) * 128],
                        ident[D:D + 1, D:D + 1])
                    rc = rpool.tile([128, 1], F32, tag="rc")
                    nc.vector.reciprocal(rc, tp)
                    recips[(h, tch)] = rc

            for tch in range(N // 128):
                g = b * (N // 128) + tch
                og = opool.tile([128, DIM], MMD, tag="og")
                for ech in range(DIM // 512):
                    pps = []
                    for h in range(HL):
                        pp = scp.tile([128, 512], F32, tag="sc")
                        nc.tensor.matmul(
                            pp,
                            uns[h][:, tch * 128:(tch + 1) * 128],
                            wp_sb[:, h, ech * 512:(ech + 1) * 512],
                            start=True, stop=True)
                        pps.append(pp)
                    s0 = opool.tile([128, 512], F32, tag="o")
                    nc.vector.tensor_scalar_mul(s0, pps[0], recips[(0, tch)])
                    s1 = opool.tile([128, 512], F32, tag="o")
                    nc.vector.tensor_scalar_mul(s1, pps[1], recips[(1, tch)])
                    # the head-sum runs on gpsimd (SBUF-only engine) to keep
                    # DVE free for its share of the exp tiles
                    nc.gpsimd.tensor_add(og[:, ech * 512:(ech + 1) * 512],
                                         s0, s1)
                nc.scalar.dma_start(
                    out=t_out[g * 128:(g + 1) * 128, :], in_=og)

    repeat = aps["repeat"]
    if repeat > 1:
        with tc.For_i(0, repeat, 1):
            body()
    else:
        body()


def build(repeat: int = 1, ablate: str = ""):
    """Build + bass-compile the SPMD program (one NeuronCore's view)."""
    nc = bacc.Bacc("TRN2", target_bir_lowering=False, debug=False,
                   num_devices=N_CORES)
    aps = {
        "x_t": nc.dram_tensor("x_t", [DIM, T], MMD, kind="ExternalInput").ap(),
        "k_t": nc.dram_tensor("k_t", [B, HL, D, L], MMD, kind="ExternalInput").ap(),
        "v_pad": nc.dram_tensor("v_pad", [B, HL, 128, L // NBLK, VP], MMD, kind="ExternalInput").ap(),
        "w_loc": nc.dram_tensor("w_loc", [DIM, 3 * DL], MMD, kind="ExternalInput").ap(),
        "b_q": nc.dram_tensor("b_qk", [DL, 2], F32, kind="ExternalInput").ap(),
        "b_v": nc.dram_tensor("b_v", [DL], F32, kind="ExternalInput").ap(),
        "wp_loc": nc.dram_tensor("wp_loc", [D, HL, DIM], MMD, kind="ExternalInput").ap(),
        "out": nc.dram_tensor("out", [T, DIM], MMD, kind="ExternalOutput").ap(),
        "repeat": repeat,
        "ablate": ablate,
    }
    with tile.TileContext(nc) as tc:
        with ExitStack() as ctx:
            _emit(ctx, tc, aps)
    nc.compile()
    return nc


def shard_inputs(x, past_k, past_v, w_qkv, b_qkv, w_proj):
    """Full inputs -> list of 8 per-core input maps (head-sharded, bf16)."""
    import ml_dtypes
    BF16 = ml_dtypes.bfloat16

    x = np.asarray(x, np.float32)
    past_k = np.asarray(past_k, np.float32)
    past_v = np.asarray(past_v, np.float32)
    w_qkv = np.asarray(w_qkv, np.float32)
    b_qkv = np.asarray(b_qkv, np.float32)
    w_proj = np.asarray(w_proj, np.float32)

    x_t = np.ascontiguousarray(x.reshape(T, DIM).T.astype(BF16))
    in_maps = []
    for c in range(N_CORES):
        lo, hi = c * DL, (c + 1) * DL
        k_t = np.ascontiguousarray(
            past_k[:, c * HL:(c + 1) * HL].transpose(0, 1, 3, 2).astype(BF16))
        # [B, HL, L, D] -> [B, HL, 128, L//128, D+1] with a ones column at
        # d=D (softmax-denominator trick): partition dim = l % 128, one
        # fully contiguous per-partition run per DMA
        v_blk = (past_v[:, c * HL:(c + 1) * HL]
                 .reshape(B, HL, L // NBLK, NBLK, D).transpose(0, 1, 3, 2, 4))
        v_pad = np.ones((B, HL, NBLK, L // NBLK, VP), dtype=BF16)
        v_pad[..., :D] = v_blk.astype(BF16)
        v_pad = np.ascontiguousarray(v_pad)
        w_loc = np.ascontiguousarray(np.concatenate(
            [w_qkv[:, lo:hi], w_qkv[:, DIM + lo:DIM + hi],
             w_qkv[:, 2 * DIM + lo:2 * DIM + hi]], axis=1).astype(BF16))
        in_maps.append({
            "x_t": x_t,
            "k_t": k_t,
            "v_pad": v_pad,
            "w_loc": w_loc,
            "b_qk": np.ascontiguousarray(np.stack(
                [b_qkv[lo:hi], b_qkv[DIM + lo:DIM + hi]], axis=1)),
            "b_v": np.ascontiguousarray(b_qkv[2 * DIM + lo:2 * DIM + hi]),
            "wp_loc": np.ascontiguousarray(
                w_proj[lo:hi].reshape(HL, D, DIM).transpose(1, 0, 2)
                .astype(BF16)),
        })
    return in_maps


_NC_CACHE = {}


def get_nc(repeat: int = 1, ablate: str = ""):
    key = (repeat, MMD, ablate)
    if key not in _NC_CACHE:
        _NC_CACHE[key] = build(repeat, ablate)
    return _NC_CACHE[key]


def kernel(x, past_k, past_v, w_qkv, b_qkv, w_proj, b_proj):
    in_maps = shard_inputs(x, past_k, past_v, w_qkv, b_qkv, w_proj)
    nc = get_nc(1)
    try:
        res = run_bass_kernel_spmd(nc, in_maps, core_ids=list(range(N_CORES)))
    except Exception:
        # transient NRT_EXEC_UNIT_UNRECOVERABLE has been observed once on
        # this setup; a plain retry recovers it
        res = run_bass_kernel_spmd(nc, in_maps, core_ids=list(range(N_CORES)))
    out = np.zeros((T, DIM), np.float64)
    for c in range(N_CORES):
        out += np.asarray(res.results[c]["out"]).astype(np.float32)
    out += np.asarray(b_proj, np.float32)
    return out.reshape(B, N, DIM).astype(np.float32)



# revision 24
# speedup vs baseline: 1.0243x; 1.0243x over previous
"""Multi-head attention with KV-cache append, sharded over 8 trn2 NeuronCores.

Reference computation (fp32):
    qkv = x @ w_qkv + b_qkv                  # x [2,256,1024]
    q,k_new,v_new -> [B,H,N,D] (H=16, D=64)
    k_all = cat(past_k, k_new); v_all = cat(past_v, v_new)   # [B,H,8448,64]
    out = softmax(q k^T / sqrt(D)) @ v_all   # non-causal
    return out.merge_heads @ w_proj + b_proj

Sharding: tensor-parallel over heads. Core c owns heads [2c, 2c+1]:
  - past_k/past_v sharded by head and converted to bf16 on the host;
    past_k pre-transposed to [B,2,D,L] and past_v re-blocked to
    [B,2,128,L/128,D] so every DMA reads fully contiguous per-partition
    runs. On-device, v tiles get a ones column appended, so the attn@v
    matmul also produces the softmax denominator row for free.
  - w_qkv column-split / w_proj row-split per head (bf16); each core
    computes a full-shape bf16 partial of the output projection; host sums
    partials in f64 + b_proj.

Device kernel per core (all engines via the Tile framework):
  q^T/k_new^T/v_new projections on PE, flash-style streaming over the KV
  cache: scores^T block [128l, 256n] on PE (two heads row-group concurrent)
  -> exp split between ACT (table exp, scale folded) and DVE (Schraudolph
  bf16 bit-trick: one tensor_scalar) -> attn@v accumulation in PSUM with
  the ones-column producing the softmax denominator row. Epilogue: per-head
  output projection, per-token denominator scaling (softmax normalization
  commutes with the projection within a head) on DVE, cross-head sum on
  gpsimd, bf16 output partials.
"""

import sys
for _p in ("/opt/trn_rl_repo", "/root/.axon_site/_ro/trn_rl_repo"):
    if _p not in sys.path:
        sys.path.append(_p)

from contextlib import ExitStack

import numpy as np

import concourse.bass as bass
import concourse.tile as tile
from concourse import bacc, mybir
from concourse.bass_utils import run_bass_kernel_spmd
from concourse.masks import make_identity

N_CORES = 8
B, N, DIM = 2, 256, 1024
H, D = 16, 64
L = 8192
HL = H // N_CORES          # 2 heads per core
DL = HL * D                # 128 local head dims
SCALE = D ** -0.5
T = B * N                  # 512 tokens
CC = DIM // 128            # 8 contraction chunks of the model dim
F32 = mybir.dt.float32
VP = D + 1                 # v padded with ones column
NBLK = 128                 # kv-cache block (l) per scores matmul
PSB = 4                    # l-blocks per PSUM superblock -> [128, PSB*N]
DSB = 4096                 # l per DMA superblock

# bf16 data path: halves HBM traffic vs f32 and streams the PE at 1
# cycle/row at any moving size (fp32r pays 4x below 256). The ~0.4% bf16
# rounding noise is far inside the 2e-2 harness tolerance.
MMD = mybir.dt.bfloat16
U16 = mybir.dt.uint16

# Softmax exp is ~56us of ACT work if done entirely on the scalar engine --
# the single largest serial engine cost. Offload a fraction of the exp tiles
# to the DVE using the Schraudolph bit trick in bf16:
#   bf16_bits(exp(s*SCALE)) ~= round(s * EXP_A + EXP_B)
# (one tensor_scalar mult+add, f32 PSUM in -> uint16 out, bitcast to bf16).
# Max rel err ~3.3%, rms ~1.8%, pseudo-random across keys -> averages out in
# the softmax weighted mean (common-mode cancels in the ratio entirely).
EXP_A = float(SCALE * np.log2(np.e) * 128.0)
EXP_B = 16250.5
# of every 16 exp tiles, these slot indices run on DVE (rest on ACT)
DVE_SLOTS = frozenset((1, 3, 5, 7, 9, 11, 13))


def _emit(ctx: ExitStack, tc: tile.TileContext, aps: dict):
    nc = tc.nc
    t_x, t_k, t_v = aps["x_t"], aps["k_t"], aps["v_pad"]
    t_w, t_bq, t_bv = aps["w_loc"], aps["b_q"], aps["b_v"]
    t_wp, t_out = aps["wp_loc"], aps["out"]
    ablate = aps.get("ablate", "")

    singles = ctx.enter_context(tc.tile_pool(name="singles", bufs=1))
    kpool = ctx.enter_context(tc.tile_pool(name="kpool", bufs=2))
    vpool = ctx.enter_context(tc.tile_pool(name="vpool", bufs=4))
    apool = ctx.enter_context(tc.tile_pool(name="apool", bufs=4))
    epool = ctx.enter_context(tc.tile_pool(name="epool", bufs=4))
    opool = ctx.enter_context(tc.tile_pool(name="opool", bufs=6))
    rpool = ctx.enter_context(tc.tile_pool(name="rpool", bufs=8))
    scp = ctx.enter_context(tc.tile_pool(name="scp", bufs=3, space="PSUM"))
    acp = ctx.enter_context(tc.tile_pool(name="acp", bufs=2, space="PSUM"))

    Exp = mybir.ActivationFunctionType.Exp
    Ident = mybir.ActivationFunctionType.Identity

    def body():
        # ---- load x^T, weights, biases ----
        # x/w split into halves so the qkv matmuls (which consume one
        # cc-chunk at a time) start as soon as the first half lands
        x_sb = singles.tile([128, CC, T], MMD, tag="x")
        xr = t_x.rearrange("(cc p) t -> p cc t", p=128)
        nc.sync.dma_start(out=x_sb[:, 0:CC // 2], in_=xr[:, 0:CC // 2])
        nc.sync.dma_start(out=x_sb[:, CC // 2:CC], in_=xr[:, CC // 2:CC])
        w_sb = singles.tile([128, CC, 3 * DL], MMD, tag="w")
        wr = t_w.rearrange("(cc p) d -> p cc d", p=128)
        nc.scalar.dma_start(out=w_sb[:, 0:CC // 2], in_=wr[:, 0:CC // 2])
        nc.scalar.dma_start(out=w_sb[:, CC // 2:CC], in_=wr[:, CC // 2:CC])
        wp_sb = singles.tile([D, HL, DIM], MMD, tag="wp")
        nc.scalar.dma_start(out=wp_sb, in_=t_wp)
        bqk_sb = singles.tile([DL, 2], F32, tag="bqk")
        nc.scalar.dma_start(out=bqk_sb, in_=t_bq)
        bq_sb, bk_sb = bqk_sb[:, 0:1], bqk_sb[:, 1:2]
        bv_sb = singles.tile([128, DL], F32, tag="bv")
        nc.gpsimd.dma_start(
            out=bv_sb,
            in_=bass.AP(tensor=t_bv.tensor, offset=0, ap=[[0, 128], [1, DL]]),
        )
        ident = singles.tile([128, 128], F32, tag="ident")
        make_identity(nc, ident)
        ones_sb = singles.tile([128, 1], F32, tag="ones")
        nc.vector.memset(ones_sb, 1.0)
        # touch Exp once while ACT is otherwise idle: pulls the ~2.7us
        # ACT_TABLE_LOAD for the exp table set into the startup window
        # instead of stalling the first real exp of the pipeline
        warm = singles.tile([128, 1], F32, tag="warm")
        nc.scalar.activation(warm, ones_sb, Exp)

        if ablate == "dmaonly":
            for b in range(B):
                for dsb in range(L // DSB):
                    NB = DSB // NBLK
                    kt = kpool.tile([128, DSB], MMD, tag="k")
                    nc.sync.dma_start(
                        out=kt,
                        in_=t_k[b].rearrange("h d l -> (h d) l")[:, dsb * DSB:(dsb + 1) * DSB])
                    for h in range(HL):
                        vt = vpool.tile([128, NB, VP], MMD, tag="v", name=f"vt{h}")
                        veng = nc.gpsimd if h == 0 else nc.sync
                        veng.dma_start(
                            out=vt,
                            in_=t_v[b, h][:, dsb * NB:(dsb + 1) * NB, :])
            for g in range(4):
                for ech in range(2):
                    so = opool.tile([128, 512], MMD, tag="o")
                    nc.vector.tensor_copy(so, x_sb[:, g, :])
                    nc.sync.dma_start(
                        out=t_out[g * 128:(g + 1) * 128,
                                  ech * 512:(ech + 1) * 512],
                        in_=so)
            return

        at0 = None
        if ablate in ("noact", "nopev"):
            at0 = singles.tile([128, PSB * N], MMD, tag="at0")
            nc.vector.memset(at0, 0.001)
        exp_cnt = [0]

        # ---- qkv projections ----
        # q^T, k_new^T in [d_local, token] layout (d on partitions)
        psq = scp.tile([128, T], F32, tag="sc")
        for cc in range(CC):
            nc.tensor.matmul(psq, w_sb[:, cc, 0:DL], x_sb[:, cc, :],
                             start=(cc == 0), stop=(cc == CC - 1))
        q_sb = singles.tile([DL, T], MMD, tag="q")
        nc.scalar.activation(q_sb, psq, Ident, bias=bq_sb)

        psk = scp.tile([128, T], F32, tag="sc")
        for cc in range(CC):
            nc.tensor.matmul(psk, w_sb[:, cc, DL:2 * DL], x_sb[:, cc, :],
                             start=(cc == 0), stop=(cc == CC - 1))
        k_sb = singles.tile([DL, T], MMD, tag="kn")
        nc.scalar.activation(k_sb, psk, Ident, bias=bk_sb)

        # v_new in [token, d_local] layout, padded with the ones column
        vn_sb = {}
        for nch in range(T // 128):  # 4 chunks of 128 tokens
            psv = acp.tile([128, DL], F32, tag="ac")
            for cc in range(CC):
                nc.tensor.matmul(psv, x_sb[:, cc, nch * 128:(nch + 1) * 128],
                                 w_sb[:, cc, 2 * DL:3 * DL],
                                 start=(cc == 0), stop=(cc == CC - 1))
            for h in range(HL):
                vn = singles.tile([128, VP], MMD, tag=f"vn{nch}_{h}")
                nc.vector.tensor_add(vn[:, 0:D], psv[:, h * D:(h + 1) * D],
                                     bv_sb[:, h * D:(h + 1) * D])
                nc.vector.tensor_copy(vn[:, D:VP], ones_sb)
                vn_sb[(nch, h)] = vn

        # ---- streaming attention over the KV cache ----
        for b in range(B):
            acc = {}
            for h in range(HL):
                acc[(b, h)] = acp.tile([VP, N], F32, tag="ac", name=f"acc{b}{h}")

            # software pipeline: the attn@v of superblock i is emitted after
            # the scores of superblock i+1 so the (strict-FIFO) PE never
            # stalls waiting for the exp of the tile it just produced
            pend = []

            def flush_attnv():
                while pend:
                    p_ats, p_vts, base = pend.pop(0)
                    for j in range(PSB):
                        for h in range(HL):
                            if ablate == "nopev":
                                continue
                            nc.tensor.matmul(
                                acc[(b, h)],
                                p_vts[h][:, base % (DSB // NBLK) + j, :],
                                p_ats[h][:, j * N:(j + 1) * N],
                                start=(base + j == 0), stop=False,
                                skip_group_check=True)
            for dsb in range(L // DSB):  # DMA superblocks
                l0 = dsb * DSB
                NB = DSB // NBLK
                kt = kpool.tile([128, DSB], MMD, tag="k")
                nc.sync.dma_start(
                    out=kt,
                    in_=t_k[b].rearrange("h d l -> (h d) l")[:, l0:l0 + DSB])
                vts = {}
                for h in range(HL):
                    # v is host-laid-out as [128, L//128, D+1] with the ones
                    # column baked in: partition = l within a 128-block ->
                    # one fully contiguous per-partition DMA run (>=512B, no
                    # SDMA read-modify-write), and no device-side ones copy.
                    # The two heads ride different DMA queues.
                    vt = vpool.tile([128, NB, VP], MMD, tag="v", name=f"vt{h}")
                    # heavy mid-kernel DMAs only on queues of engines that
                    # are idle enough to issue doorbells promptly (SP,
                    # gpsimd) -- the ACT queue would issue them late from
                    # behind its exp backlog
                    veng = nc.gpsimd if h == 0 else nc.sync
                    veng.dma_start(
                        out=vt,
                        in_=t_v[b, h][:, dsb * NB:(dsb + 1) * NB, :])
                    vts[h] = vt
                for psb in range(DSB // (PSB * NBLK)):
                    # the two heads' scores matmuls sit on disjoint PE
                    # row-groups (partitions 0-63 / 64-127): interleaving
                    # them lets the array run both concurrently
                    pss, ats = {}, {}
                    for h in range(HL):
                        pss[h] = scp.tile([128, PSB * N], F32, tag="sc",
                                          name=f"ps{h}")
                    for j in range(PSB):
                        lo = psb * PSB * NBLK + j * NBLK
                        for h in range(HL):
                            nc.tensor.matmul(
                                pss[h][:, j * N:(j + 1) * N],
                                kt[h * D:(h + 1) * D, lo:lo + NBLK],
                                q_sb[h * D:(h + 1) * D, b * N:(b + 1) * N],
                                start=True, stop=True)
                    for h in range(HL):
                        if ablate == "noact":
                            ats[h] = at0
                            continue
                        slot = exp_cnt[0] % 16
                        exp_cnt[0] += 1
                        if slot in DVE_SLOTS:
                            atu = apool.tile([128, PSB * N], U16, tag="a",
                                             name=f"at{h}")
                            nc.vector.tensor_scalar(
                                atu, pss[h], EXP_A, EXP_B,
                                mybir.AluOpType.mult, mybir.AluOpType.add)
                            ats[h] = atu.bitcast(MMD)
                        else:
                            at = apool.tile([128, PSB * N], MMD, tag="a",
                                            name=f"at{h}")
                            nc.scalar.activation(at, pss[h], Exp, scale=SCALE)
                            ats[h] = at
                    prev = pend
                    pend = [(ats, vts, dsb * (DSB // NBLK) + psb * PSB)]
                    while prev:
                        p_ats, p_vts, base = prev.pop(0)
                        for j in range(PSB):
                            for h in range(HL):
                                if ablate == "nopev":
                                    continue
                                nc.tensor.matmul(
                                    acc[(b, h)],
                                    p_vts[h][:, base % (DSB // NBLK) + j, :],
                                    p_ats[h][:, j * N:(j + 1) * N],
                                    start=(base + j == 0), stop=False,
                                    skip_group_check=True)
            flush_attnv()
            # new tokens (the appended k_new/v_new of this batch)
            for h in range(HL):
                ps = scp.tile([128, 2 * N], F32, tag="sc")
                for j in range(2):
                    nc.tensor.matmul(
                        ps[:, j * N:(j + 1) * N],
                        k_sb[h * D:(h + 1) * D,
                                b * N + j * NBLK:b * N + (j + 1) * NBLK],
                        q_sb[h * D:(h + 1) * D, b * N:(b + 1) * N],
                        start=True, stop=True)
                at = apool.tile([128, 2 * N], MMD, tag="a")
                nc.scalar.activation(at, ps, Exp, scale=SCALE)
                for j in range(2):
                    nc.tensor.matmul(
                        acc[(b, h)], vn_sb[(b * 2 + j, h)],
                        at[:, j * N:(j + 1) * N],
                        start=(ablate == "nopev" and j == 0),
                        stop=(j == 1), skip_group_check=True)

            # ---- per-batch epilogue: denominators, per-head projection,
            # normalize+sum (softmax normalization commutes with the
            # projection within a head) ----
            uns, recips = {}, {}
            for h in range(HL):
                # attn-out (unnormalized) to SBUF bf16 for the oproj lhsT
                un = epool.tile([D, N], MMD, tag="un")
                nc.vector.tensor_copy(un, acc[(b, h)][0:D, :])
                uns[h] = un
                # denominator row stays f32: copy out (partition-aligned at
                # row 64), transpose each 128-token chunk to a per-partition
                # column, reciprocal
                den = epool.tile([VP, N], F32, tag="den")
                nc.vector.tensor_copy(den[D:VP, :], acc[(b, h)][D:VP, :])
                for tch in range(N // 128):
                    tp = scp.tile([128, 1], F32, tag="sc")
                    nc.tensor.transpose(
                        tp, den[D:VP, tch * 128:(tch + 1) * 128],
                        ident[D:D + 1, D:D + 1])
                    rc = rpool.tile([128, 1], F32, tag="rc")
                    nc.vector.reciprocal(rc, tp)
                    recips[(h, tch)] = rc

            for tch in range(N // 128):
                g = b * (N // 128) + tch
                og = opool.tile([128, DIM], MMD, tag="og")
                for ech in range(DIM // 512):
                    pps = []
                    for h in range(HL):
                        pp = scp.tile([128, 512], F32, tag="sc")
                        nc.tensor.matmul(
                            pp,
                            uns[h][:, tch * 128:(tch + 1) * 128],
                            wp_sb[:, h, ech * 512:(ech + 1) * 512],
                            start=True, stop=True)
                        pps.append(pp)
                    s0 = opool.tile([128, 512], F32, tag="o")
                    nc.vector.tensor_scalar_mul(s0, pps[0], recips[(0, tch)])
                    s1 = opool.tile([128, 512], F32, tag="o")
                    nc.vector.tensor_scalar_mul(s1, pps[1], recips[(1, tch)])
                    # the head-sum runs on gpsimd (SBUF-only engine) to keep
                    # DVE free for its share of the exp tiles
                    nc.gpsimd.tensor_add(og[:, ech * 512:(ech + 1) * 512],
                                         s0, s1)
                nc.sync.dma_start(
                    out=t_out[g * 128:(g + 1) * 128, :], in_=og)

    repeat = aps["repeat"]
    if repeat > 1:
        with tc.For_i(0, repeat, 1):
            body()
    else:
        body()


def build(repeat: int = 1, ablate: str = ""):
    """Build + bass-compile the SPMD program (one NeuronCore's view)."""
    nc = bacc.Bacc("TRN2", target_bir_lowering=False, debug=False,
                   num_devices=N_CORES)
    aps = {
        "x_t": nc.dram_tensor("x_t", [DIM, T], MMD, kind="ExternalInput").ap(),
        "k_t": nc.dram_tensor("k_t", [B, HL, D, L], MMD, kind="ExternalInput").ap(),
        "v_pad": nc.dram_tensor("v_pad", [B, HL, 128, L // NBLK, VP], MMD, kind="ExternalInput").ap(),
        "w_loc": nc.dram_tensor("w_loc", [DIM, 3 * DL], MMD, kind="ExternalInput").ap(),
        "b_q": nc.dram_tensor("b_qk", [DL, 2], F32, kind="ExternalInput").ap(),
        "b_v": nc.dram_tensor("b_v", [DL], F32, kind="ExternalInput").ap(),
        "wp_loc": nc.dram_tensor("wp_loc", [D, HL, DIM], MMD, kind="ExternalInput").ap(),
        "out": nc.dram_tensor("out", [T, DIM], MMD, kind="ExternalOutput").ap(),
        "repeat": repeat,
        "ablate": ablate,
    }
    with tile.TileContext(nc) as tc:
        with ExitStack() as ctx:
            _emit(ctx, tc, aps)
    nc.compile()
    return nc


def shard_inputs(x, past_k, past_v, w_qkv, b_qkv, w_proj):
    """Full inputs -> list of 8 per-core input maps (head-sharded, bf16)."""
    import ml_dtypes
    BF16 = ml_dtypes.bfloat16

    x = np.asarray(x, np.float32)
    past_k = np.asarray(past_k, np.float32)
    past_v = np.asarray(past_v, np.float32)
    w_qkv = np.asarray(w_qkv, np.float32)
    b_qkv = np.asarray(b_qkv, np.float32)
    w_proj = np.asarray(w_proj, np.float32)

    x_t = np.ascontiguousarray(x.reshape(T, DIM).T.astype(BF16))
    in_maps = []
    for c in range(N_CORES):
        lo, hi = c * DL, (c + 1) * DL
        k_t = np.ascontiguousarray(
            past_k[:, c * HL:(c + 1) * HL].transpose(0, 1, 3, 2).astype(BF16))
        # [B, HL, L, D] -> [B, HL, 128, L//128, D+1] with a ones column at
        # d=D (softmax-denominator trick): partition dim = l % 128, one
        # fully contiguous per-partition run per DMA
        v_blk = (past_v[:, c * HL:(c + 1) * HL]
                 .reshape(B, HL, L // NBLK, NBLK, D).transpose(0, 1, 3, 2, 4))
        v_pad = np.ones((B, HL, NBLK, L // NBLK, VP), dtype=BF16)
        v_pad[..., :D] = v_blk.astype(BF16)
        v_pad = np.ascontiguousarray(v_pad)
        w_loc = np.ascontiguousarray(np.concatenate(
            [w_qkv[:, lo:hi], w_qkv[:, DIM + lo:DIM + hi],
             w_qkv[:, 2 * DIM + lo:2 * DIM + hi]], axis=1).astype(BF16))
        in_maps.append({
            "x_t": x_t,
            "k_t": k_t,
            "v_pad": v_pad,
            "w_loc": w_loc,
            "b_qk": np.ascontiguousarray(np.stack(
                [b_qkv[lo:hi], b_qkv[DIM + lo:DIM + hi]], axis=1)),
            "b_v": np.ascontiguousarray(b_qkv[2 * DIM + lo:2 * DIM + hi]),
            "wp_loc": np.ascontiguousarray(
                w_proj[lo:hi].reshape(HL, D, DIM).transpose(1, 0, 2)
                .astype(BF16)),
        })
    return in_maps


_NC_CACHE = {}


def get_nc(repeat: int = 1, ablate: str = ""):
    key = (repeat, MMD, ablate)
    if key not in _NC_CACHE:
        _NC_CACHE[key] = build(repeat, ablate)
    return _NC_CACHE[key]


def kernel(x, past_k, past_v, w_qkv, b_qkv, w_proj, b_proj):
    in_maps = shard_inputs(x, past_k, past_v, w_qkv, b_qkv, w_proj)
    nc = get_nc(1)
    try:
        res = run_bass_kernel_spmd(nc, in_maps, core_ids=list(range(N_CORES)))
    except Exception:
        # transient NRT_EXEC_UNIT_UNRECOVERABLE has been observed once on
        # this setup; a plain retry recovers it
        res = run_bass_kernel_spmd(nc, in_maps, core_ids=list(range(N_CORES)))
    out = np.zeros((T, DIM), np.float64)
    for c in range(N_CORES):
        out += np.asarray(res.results[c]["out"]).astype(np.float32)
    out += np.asarray(b_proj, np.float32)
    return out.reshape(B, N, DIM).astype(np.float32)

